# revision 15
# baseline (speedup 1.0000x reference)
"""Trainium2 Bass kernel: batched masked attention (B=8, S=2048, D=512).

Data-parallel over batch: one batch element per NeuronCore (8 cores).
Per core:
  Q^T = (Wq @ x^T + bq)/sqrt(D), K^T = Wk @ x^T + bk   (transposed layout, bf16)
  V   = x @ Wv^T + bv                                   (natural layout, bf16)
  s   = Q K^T (fp32 PSUM)   e = exp(s) * mask (bf16)    sum = row-sum(e)
  weights = e / sum (fp32 out)   Sout = (e @ V) / sum (fp32 out)
No max-subtraction needed: scores are ~N(0,1), |s| < ~8, exp is safe in fp32,
and exp(s)*mask == softmax numerator since the mask multiply zeroes masked
entries exactly.
"""

import os
import sys
import math

import numpy as np

for _p in ("/opt/trn_rl_repo",):
    if _p not in sys.path:
        sys.path.insert(0, _p)

import concourse.bass as bass
import concourse.mybir as mybir
import concourse.tile as tile
from concourse import bacc
from concourse.bass_utils import run_bass_kernel_spmd
from concourse.masks import make_identity

B, S, D = 8, 2048, 512
N_CORES = 8
P = 128          # SBUF partitions
DC = D // P      # 4 chunks of the d_model contraction
ST = S // P      # 16 seq tiles of 128
FB = 512         # matmul moving free-dim block (one PSUM bank of fp32)
NB = S // FB     # 4 free blocks across seq

F32 = mybir.dt.float32
BF16 = mybir.dt.bfloat16
I32 = mybir.dt.int32
AF = mybir.ActivationFunctionType
ALU = mybir.AluOpType

LAST = {"exec_time_ns": None, "results": None}
_cache = {}
ET_DMA = bool(os.environ.get("KERNEL_ET_DMA"))


def _body(nc, tc, x_ext, mask_ext, W_ext, b_ext, out_s, out_w):
    inv_sqrt_d = 1.0 / math.sqrt(D)

    const = tc.alloc_tile_pool(name="const", bufs=1)
    big = tc.alloc_tile_pool(name="big", bufs=1)

    id_f32 = const.tile([P, P], F32, tag="id_f32")
    make_identity(nc, id_f32)
    id_bf16 = const.tile([P, P], BF16, tag="id_bf16")
    make_identity(nc, id_bf16)

    # ---- biases ----
    # b as [P, DC]: column c holds b[c*128 : (c+1)*128]
    bq_s = const.tile([P, DC], F32, tag="bq_s")
    bk_t = const.tile([P, DC], F32, tag="bk_t")
    bq_raw = const.tile([P, DC], F32, tag="bq_raw")
    nc.gpsimd.dma_start(out=bq_raw, in_=b_ext["q"].rearrange("(c p) -> p c", c=DC))
    nc.vector.tensor_scalar_mul(bq_s, bq_raw, inv_sqrt_d)
    nc.gpsimd.dma_start(out=bk_t, in_=b_ext["k"].rearrange("(c p) -> p c", c=DC))
    # b_v broadcast across partitions: [P, D]
    bv_bcast = const.tile([P, D], F32, tag="bv_bcast")
    nc.gpsimd.dma_start(out=bv_bcast, in_=b_ext["v"].partition_broadcast(P))

    # ---- persistent bf16 operands ----
    xT = [big.tile([P, S], BF16, tag=f"xT{c}", name=f"xT{c}") for c in range(DC)]
    QT = [big.tile([P, S], BF16, tag=f"QT{c}", name=f"QT{c}") for c in range(DC)]
    KT = [big.tile([P, S], BF16, tag=f"KT{c}", name=f"KT{c}") for c in range(DC)]
    WT = {m: [big.tile([P, D], BF16, tag=f"WT{m}{c}", name=f"WT{m}{c}") for c in range(DC)]
          for m in "qkv"}
    V = [big.tile([P, FB], BF16, tag=f"V{st}", name=f"V{st}") for st in range(ST)]

    # =================== prep phase ===================
    with (
        tc.tile_pool(name="wload", bufs=5) as wload,
        tc.tile_pool(name="xload", bufs=6) as xload,
        tc.tile_pool(name="psum_prep", bufs=6, space="PSUM") as pp,
    ):
        # --- PE warmup: dummy transposes to flip the HAM clock gate to 8/8
        # while the first DMAs are in flight ---
        warm = pp.tile([P, P], BF16, tag="warm", bufs=1, name="warm")
        for _ in range(48):
            nc.tensor.transpose(warm, id_bf16, id_bf16)

        # --- W^T: load W[m] (4 row tiles of [128, 512]) and PE-transpose ---
        for m in "qkv":
            wtiles = []
            for rt in range(DC):
                wt = wload.tile([P, D], F32, tag=f"wt{rt}", name=f"wt{rt}")
                nc.sync.dma_start(out=wt, in_=W_ext[m][rt * P:(rt + 1) * P, :])
                wtiles.append(wt)
            for c in range(DC):
                ps = pp.tile([P, D], F32, tag="pp", name="pw")
                for rt in range(DC):
                    nc.tensor.transpose(
                        ps[:, rt * P:(rt + 1) * P],
                        wtiles[rt][:, c * P:(c + 1) * P],
                        id_f32,
                    )
                nc.scalar.activation(out=WT[m][c], in_=ps, func=AF.Copy)

        # --- x^T + projections, interleaved per block of 4 seq tiles so the
        # PE can start projections before all of x is transposed ---
        for g in range(ST // 4):
            xtiles = []
            for j in range(4):
                st = g * 4 + j
                xt = xload.tile([P, D], F32, tag=f"xload{st % 6}", name=f"xl{st}")
                nc.sync.dma_start(out=xt, in_=x_ext[st * P:(st + 1) * P, :])
                xtiles.append(xt)
            for c in range(DC):
                ps = pp.tile([P, D], F32, tag="pp", name="px")
                for j in range(4):
                    nc.tensor.transpose(
                        ps[:, j * P:(j + 1) * P],
                        xtiles[j][:, c * P:(c + 1) * P],
                        id_f32,
                    )
                nc.scalar.activation(
                    out=xT[c][:, g * D:(g + 1) * D], in_=ps, func=AF.Copy
                )
            # projections that only need xT columns of this block (qb = g)
            qb = g
            for m, dst, scale, bias in (
                ("q", QT, inv_sqrt_d, bq_s),
                ("k", KT, 1.0, bk_t),
            ):
                for c_out in range(DC):
                    ps = pp.tile([P, FB], F32, tag="pp", name=f"p{m}{c_out}")
                    for c_in in range(DC):
                        nc.tensor.matmul(
                            ps,
                            lhsT=WT[m][c_in][:, c_out * P:(c_out + 1) * P],
                            rhs=xT[c_in][:, qb * FB:(qb + 1) * FB],
                            start=(c_in == 0),
                            stop=(c_in == DC - 1),
                        )
                    nc.scalar.activation(
                        out=dst[c_out][:, qb * FB:(qb + 1) * FB],
                        in_=ps,
                        func=AF.Identity,
                        scale=scale,
                        bias=bias[:, c_out:c_out + 1],
                    )
            for j in range(4):
                st = g * 4 + j
                ps = pp.tile([P, FB], F32, tag="pp", name=f"pv{st}")
                for c_in in range(DC):
                    nc.tensor.matmul(
                        ps,
                        lhsT=xT[c_in][:, st * P:(st + 1) * P],
                        rhs=WT["v"][c_in],
                        start=(c_in == 0),
                        stop=(c_in == DC - 1),
                    )
                nc.vector.tensor_add(V[st], ps, bv_bcast)

    # =================== main loop over q tiles ===================
    with (
        tc.tile_pool(name="work", bufs=3) as work,
        tc.tile_pool(name="stat", bufs=3) as stat,
        tc.tile_pool(name="psum_s", bufs=1, space="PSUM") as psum_s,
        tc.tile_pool(name="psum_t", bufs=2, space="PSUM") as psum_t,
        tc.tile_pool(name="psum_o", bufs=2, space="PSUM") as psum_o,
    ):
        for qt in range(ST):
            rows = slice(qt * P, (qt + 1) * P)

            # mask load with int32 -> bf16 cast (SWDGE)
            mtile = work.tile([P, S], BF16, tag="mask")
            nc.gpsimd.dma_start(out=mtile, in_=mask_ext[rows, :])

            # scores: s[kb] = Q K^T / sqrt(d) (scale folded into Q)
            pss = []
            for kb in range(NB):
                ps = psum_s.tile([P, FB], F32, tag=f"ps{kb}", name=f"ps{kb}")
                pss.append(ps)
            for c in range(DC):
                for kb in range(NB):
                    nc.tensor.matmul(
                        pss[kb],
                        lhsT=QT[c][:, rows],
                        rhs=KT[c][:, kb * FB:(kb + 1) * FB],
                        start=(c == 0),
                        stop=(c == DC - 1),
                    )

            # exp (no max subtraction; scores are O(1))
            e_raw = work.tile([P, S], BF16, tag="eraw")
            for kb in range(NB):
                nc.scalar.activation(
                    out=e_raw[:, kb * FB:(kb + 1) * FB], in_=pss[kb], func=AF.Exp
                )

            # masked numerator + row sum
            # (tensor_tensor_reduce crashes the exec unit on this runtime,
            #  so mask-multiply and row-sum are separate DVE passes)
            e = work.tile([P, S], BF16, tag="e")
            ssum = stat.tile([P, 1], F32, tag="ssum")
            nc.vector.tensor_mul(e, e_raw, mtile)
            nc.vector.tensor_reduce(ssum, e, mybir.AxisListType.X, ALU.add)
            r = stat.tile([P, 1], F32, tag="r")
            nc.vector.reciprocal(r, ssum)

            # weights output (fp32), chunked so DMA-out starts early
            wtile = work.tile([P, S], F32, tag="w")
            for kb in range(NB):
                cols = slice(kb * FB, (kb + 1) * FB)
                nc.vector.tensor_scalar_mul(wtile[:, cols], e[:, cols], r)
                nc.sync.dma_start(out=out_w[rows, cols], in_=wtile[:, cols])

            # e^T: PE transpose (default) or xbar DMA transpose (ET_DMA=1)
            eT = []
            if ET_DMA:
                for g in range(2):
                    et = work.tile([P, 8 * P], BF16, tag=f"eT{g}")
                    for j in range(8):
                        kc = g * 8 + j
                        nc.sync.dma_start(
                            out=et[:, j * P:(j + 1) * P],
                            in_=e[:, kc * P:(kc + 1) * P],
                            transpose=True,
                        )
                    eT.append(et)
            else:
                for g in range(2):
                    pt = psum_t.tile([P, 8 * P], BF16, tag="pt")
                    for j in range(8):
                        kc = g * 8 + j
                        nc.tensor.transpose(
                            pt[:, j * P:(j + 1) * P],
                            e[:, kc * P:(kc + 1) * P],
                            id_bf16,
                        )
                    et = work.tile([P, 8 * P], BF16, tag=f"eT{g}")
                    if g == 0:
                        nc.scalar.activation(out=et, in_=pt, func=AF.Copy)
                    else:
                        nc.vector.tensor_copy(out=et, in_=pt)
                    eT.append(et)

            # S_out = (e @ V) * r
            po = psum_o.tile([P, FB], F32, tag="po")
            for kc in range(ST):
                nc.tensor.matmul(
                    po,
                    lhsT=eT[kc // 8][:, (kc % 8) * P:(kc % 8 + 1) * P],
                    rhs=V[kc],
                    start=(kc == 0),
                    stop=(kc == ST - 1),
                )
            so = work.tile([P, FB], F32, tag="so")
            nc.vector.tensor_scalar_mul(so, po, r)
            nc.sync.dma_start(out=out_s[rows, :], in_=so)

    big.release()
    const.release()


def _build_nc():
    nc = bacc.Bacc("TRN2", target_bir_lowering=False, debug=False)
    x_ext = nc.dram_tensor("input", [S, D], F32, kind="ExternalInput").ap()
    mask_ext = nc.dram_tensor("mask", [S, S], I32, kind="ExternalInput").ap()
    W_ext = {m: nc.dram_tensor(f"W_{m}", [D, D], F32, kind="ExternalInput").ap()
             for m in "qkv"}
    b_ext = {m: nc.dram_tensor(f"b_{m}", [D], F32, kind="ExternalInput").ap()
             for m in "qkv"}
    out_s = nc.dram_tensor("out_s", [S, D], F32, kind="ExternalOutput").ap()
    out_w = nc.dram_tensor("out_w", [S, S], F32, kind="ExternalOutput").ap()

    with tile.TileContext(nc) as tc:
        _body(nc, tc, x_ext, mask_ext, W_ext, b_ext, out_s, out_w)
    nc.compile()
    return nc


def _get_nc():
    if "nc" not in _cache:
        _cache["nc"] = _build_nc()
    return _cache["nc"]


def kernel(**inputs):
    x = np.ascontiguousarray(np.asarray(inputs["input"], dtype=np.float32))
    mask = np.ascontiguousarray(np.asarray(inputs["mask"], dtype=np.int32))
    Ws = {m: np.ascontiguousarray(np.asarray(inputs[f"W_{m}"], dtype=np.float32))
          for m in "qkv"}
    bs = {m: np.ascontiguousarray(np.asarray(inputs[f"b_{m}"], dtype=np.float32))
          for m in "qkv"}

    nc = _get_nc()
    in_maps = []
    for i in range(N_CORES):
        m = {"input": x[i], "mask": mask[i]}
        for k in "qkv":
            m[f"W_{k}"] = Ws[k]
            m[f"b_{k}"] = bs[k]
        in_maps.append(m)

    trace = bool(os.environ.get("KERNEL_TRACE"))
    res = run_bass_kernel_spmd(
        nc, in_maps, core_ids=list(range(N_CORES)), trace=trace
    )
    LAST["exec_time_ns"] = res.exec_time_ns
    LAST["results"] = res
    Sout = np.stack([np.asarray(res.results[i]["out_s"]) for i in range(N_CORES)])
    Wout = np.stack([np.asarray(res.results[i]["out_w"]) for i in range(N_CORES)])
    return (Sout, Wout)


if __name__ == "__main__":
    rng = np.random.default_rng(0)
    ins = {
        "input": rng.standard_normal((B, S, D), dtype=np.float32),
        "mask": rng.integers(0, 2, size=(B, S, S), dtype=np.int32),
    }
    for m in "qkv":
        ins[f"W_{m}"] = rng.standard_normal((D, D), dtype=np.float32) / math.sqrt(D)
        ins[f"b_{m}"] = 0.01 * rng.standard_normal((D,), dtype=np.float32)
    out = kernel(**ins)
    print("Sout", out[0].shape, "W", out[1].shape)


# revision 16
# speedup vs baseline: 1.0057x; 1.0057x over previous
"""Trainium2 Bass kernel: batched masked attention (B=8, S=2048, D=512).

Data-parallel over batch: one batch element per NeuronCore (8 cores).
Per core:
  Q^T = (Wq @ x^T + bq)/sqrt(D), K^T = Wk @ x^T + bk   (transposed layout, bf16)
  V   = x @ Wv^T + bv                                   (natural layout, bf16)
  s   = Q K^T (fp32 PSUM)   e = exp(s) * mask (bf16)    sum = row-sum(e)
  weights = e / sum (fp32 out)   Sout = (e @ V) / sum (fp32 out)
No max-subtraction needed: scores are ~N(0,1), |s| < ~8, exp is safe in fp32,
and exp(s)*mask == softmax numerator since the mask multiply zeroes masked
entries exactly.
"""

import os
import sys
import math

import numpy as np

for _p in ("/opt/trn_rl_repo",):
    if _p not in sys.path:
        sys.path.insert(0, _p)

import concourse.bass as bass
import concourse.mybir as mybir
import concourse.tile as tile
from concourse import bacc
from concourse.bass_utils import run_bass_kernel_spmd
from concourse.masks import make_identity

B, S, D = 8, 2048, 512
N_CORES = 8
P = 128          # SBUF partitions
DC = D // P      # 4 chunks of the d_model contraction
ST = S // P      # 16 seq tiles of 128
FB = 512         # matmul moving free-dim block (one PSUM bank of fp32)
NB = S // FB     # 4 free blocks across seq

F32 = mybir.dt.float32
BF16 = mybir.dt.bfloat16
I32 = mybir.dt.int32
AF = mybir.ActivationFunctionType
ALU = mybir.AluOpType

LAST = {"exec_time_ns": None, "results": None}
_cache = {}
ET_DMA = bool(os.environ.get("KERNEL_ET_DMA"))


def _body(nc, tc, x_ext, mask_ext, W_ext, b_ext, out_s, out_w):
    inv_sqrt_d = 1.0 / math.sqrt(D)

    const = tc.alloc_tile_pool(name="const", bufs=1)
    big = tc.alloc_tile_pool(name="big", bufs=1)

    id_f32 = const.tile([P, P], F32, tag="id_f32")
    make_identity(nc, id_f32)
    id_bf16 = const.tile([P, P], BF16, tag="id_bf16")
    make_identity(nc, id_bf16)

    # ---- biases ----
    # b as [P, DC]: column c holds b[c*128 : (c+1)*128]
    bq_s = const.tile([P, DC], F32, tag="bq_s")
    bk_t = const.tile([P, DC], F32, tag="bk_t")
    bq_raw = const.tile([P, DC], F32, tag="bq_raw")
    nc.gpsimd.dma_start(out=bq_raw, in_=b_ext["q"].rearrange("(c p) -> p c", c=DC))
    nc.vector.tensor_scalar_mul(bq_s, bq_raw, inv_sqrt_d)
    nc.gpsimd.dma_start(out=bk_t, in_=b_ext["k"].rearrange("(c p) -> p c", c=DC))
    # b_v broadcast across partitions: [P, D]
    bv_bcast = const.tile([P, D], F32, tag="bv_bcast")
    nc.gpsimd.dma_start(out=bv_bcast, in_=b_ext["v"].partition_broadcast(P))

    # ---- persistent bf16 operands ----
    xT = [big.tile([P, S], BF16, tag=f"xT{c}", name=f"xT{c}") for c in range(DC)]
    QT = [big.tile([P, S], BF16, tag=f"QT{c}", name=f"QT{c}") for c in range(DC)]
    KT = [big.tile([P, S], BF16, tag=f"KT{c}", name=f"KT{c}") for c in range(DC)]
    WT = {m: [big.tile([P, D], BF16, tag=f"WT{m}{c}", name=f"WT{m}{c}") for c in range(DC)]
          for m in "qkv"}
    V = [big.tile([P, FB], BF16, tag=f"V{st}", name=f"V{st}") for st in range(ST)]

    # =================== prep phase ===================
    with (
        tc.tile_pool(name="wload", bufs=5) as wload,
        tc.tile_pool(name="xload", bufs=6) as xload,
        tc.tile_pool(name="psum_prep", bufs=6, space="PSUM") as pp,
    ):
        # --- PE warmup: dummy transposes to flip the HAM clock gate to 8/8
        # while the first DMAs are in flight ---
        warm = pp.tile([P, P], BF16, tag="warm", bufs=1, name="warm")
        for _ in range(72):
            nc.tensor.transpose(warm, id_bf16, id_bf16)

        # --- W^T: load W[m] (4 row tiles of [128, 512]) and PE-transpose ---
        for m in "qkv":
            wtiles = []
            for rt in range(DC):
                wt = wload.tile([P, D], F32, tag=f"wt{rt}", name=f"wt{rt}")
                nc.sync.dma_start(out=wt, in_=W_ext[m][rt * P:(rt + 1) * P, :])
                wtiles.append(wt)
            for c in range(DC):
                ps = pp.tile([P, D], F32, tag="pp", name="pw")
                for rt in range(DC):
                    nc.tensor.transpose(
                        ps[:, rt * P:(rt + 1) * P],
                        wtiles[rt][:, c * P:(c + 1) * P],
                        id_f32,
                    )
                nc.scalar.activation(out=WT[m][c], in_=ps, func=AF.Copy)

        # --- x^T + projections, interleaved per block of 4 seq tiles so the
        # PE can start projections before all of x is transposed ---
        for g in range(ST // 4):
            xtiles = []
            for j in range(4):
                st = g * 4 + j
                xt = xload.tile([P, D], F32, tag=f"xload{st % 6}", name=f"xl{st}")
                nc.sync.dma_start(out=xt, in_=x_ext[st * P:(st + 1) * P, :])
                xtiles.append(xt)
            for c in range(DC):
                ps = pp.tile([P, D], F32, tag="pp", name="px")
                for j in range(4):
                    nc.tensor.transpose(
                        ps[:, j * P:(j + 1) * P],
                        xtiles[j][:, c * P:(c + 1) * P],
                        id_f32,
                    )
                nc.scalar.activation(
                    out=xT[c][:, g * D:(g + 1) * D], in_=ps, func=AF.Copy
                )
            # projections that only need xT columns of this block (qb = g)
            qb = g
            for m, dst, scale, bias in (
                ("q", QT, inv_sqrt_d, bq_s),
                ("k", KT, 1.0, bk_t),
            ):
                for c_out in range(DC):
                    ps = pp.tile([P, FB], F32, tag="pp", name=f"p{m}{c_out}")
                    for c_in in range(DC):
                        nc.tensor.matmul(
                            ps,
                            lhsT=WT[m][c_in][:, c_out * P:(c_out + 1) * P],
                            rhs=xT[c_in][:, qb * FB:(qb + 1) * FB],
                            start=(c_in == 0),
                            stop=(c_in == DC - 1),
                        )
                    nc.scalar.activation(
                        out=dst[c_out][:, qb * FB:(qb + 1) * FB],
                        in_=ps,
                        func=AF.Identity,
                        scale=scale,
                        bias=bias[:, c_out:c_out + 1],
                    )
            for j in range(4):
                st = g * 4 + j
                ps = pp.tile([P, FB], F32, tag="pp", name=f"pv{st}")
                for c_in in range(DC):
                    nc.tensor.matmul(
                        ps,
                        lhsT=xT[c_in][:, st * P:(st + 1) * P],
                        rhs=WT["v"][c_in],
                        start=(c_in == 0),
                        stop=(c_in == DC - 1),
                    )
                nc.vector.tensor_add(V[st], ps, bv_bcast)

    # =================== main loop over q tiles ===================
    with (
        tc.tile_pool(name="work", bufs=3) as work,
        tc.tile_pool(name="stat", bufs=3) as stat,
        tc.tile_pool(name="psum_s", bufs=1, space="PSUM") as psum_s,
        tc.tile_pool(name="psum_t", bufs=2, space="PSUM") as psum_t,
        tc.tile_pool(name="psum_o", bufs=2, space="PSUM") as psum_o,
    ):
        for qt in range(ST):
            rows = slice(qt * P, (qt + 1) * P)

            # mask load with int32 -> bf16 cast (SWDGE)
            mtile = work.tile([P, S], BF16, tag="mask")
            nc.gpsimd.dma_start(out=mtile, in_=mask_ext[rows, :])

            # scores: s[kb] = Q K^T / sqrt(d) (scale folded into Q)
            pss = []
            for kb in range(NB):
                ps = psum_s.tile([P, FB], F32, tag=f"ps{kb}", name=f"ps{kb}")
                pss.append(ps)
            for c in range(DC):
                for kb in range(NB):
                    nc.tensor.matmul(
                        pss[kb],
                        lhsT=QT[c][:, rows],
                        rhs=KT[c][:, kb * FB:(kb + 1) * FB],
                        start=(c == 0),
                        stop=(c == DC - 1),
                    )

            # exp (no max subtraction; scores are O(1))
            e_raw = work.tile([P, S], BF16, tag="eraw")
            for kb in range(NB):
                nc.scalar.activation(
                    out=e_raw[:, kb * FB:(kb + 1) * FB], in_=pss[kb], func=AF.Exp
                )

            # masked numerator + row sum
            # (tensor_tensor_reduce crashes the exec unit on this runtime,
            #  so mask-multiply and row-sum are separate DVE passes)
            e = work.tile([P, S], BF16, tag="e")
            ssum = stat.tile([P, 1], F32, tag="ssum")
            nc.vector.tensor_mul(e, e_raw, mtile)
            nc.vector.tensor_reduce(ssum, e, mybir.AxisListType.X, ALU.add)
            r = stat.tile([P, 1], F32, tag="r")
            nc.vector.reciprocal(r, ssum)

            # weights output (fp32), chunked so DMA-out starts early
            wtile = work.tile([P, S], F32, tag="w")
            for kb in range(NB):
                cols = slice(kb * FB, (kb + 1) * FB)
                nc.vector.tensor_scalar_mul(wtile[:, cols], e[:, cols], r)
                nc.sync.dma_start(out=out_w[rows, cols], in_=wtile[:, cols])

            # e^T: PE transpose (default) or xbar DMA transpose (ET_DMA=1)
            eT = []
            if ET_DMA:
                for g in range(2):
                    et = work.tile([P, 8 * P], BF16, tag=f"eT{g}")
                    for j in range(8):
                        kc = g * 8 + j
                        nc.sync.dma_start(
                            out=et[:, j * P:(j + 1) * P],
                            in_=e[:, kc * P:(kc + 1) * P],
                            transpose=True,
                        )
                    eT.append(et)
            else:
                for g in range(2):
                    pt = psum_t.tile([P, 8 * P], BF16, tag="pt")
                    for j in range(8):
                        kc = g * 8 + j
                        nc.tensor.transpose(
                            pt[:, j * P:(j + 1) * P],
                            e[:, kc * P:(kc + 1) * P],
                            id_bf16,
                        )
                    et = work.tile([P, 8 * P], BF16, tag=f"eT{g}")
                    nc.vector.tensor_copy(out=et, in_=pt)
                    eT.append(et)

            # S_out = (e @ V) * r
            po = psum_o.tile([P, FB], F32, tag="po")
            for kc in range(ST):
                nc.tensor.matmul(
                    po,
                    lhsT=eT[kc // 8][:, (kc % 8) * P:(kc % 8 + 1) * P],
                    rhs=V[kc],
                    start=(kc == 0),
                    stop=(kc == ST - 1),
                )
            so = work.tile([P, FB], F32, tag="so")
            nc.vector.tensor_scalar_mul(so, po, r)
            nc.sync.dma_start(out=out_s[rows, :], in_=so)

    big.release()
    const.release()


def _build_nc():
    nc = bacc.Bacc("TRN2", target_bir_lowering=False, debug=False)
    x_ext = nc.dram_tensor("input", [S, D], F32, kind="ExternalInput").ap()
    mask_ext = nc.dram_tensor("mask", [S, S], I32, kind="ExternalInput").ap()
    W_ext = {m: nc.dram_tensor(f"W_{m}", [D, D], F32, kind="ExternalInput").ap()
             for m in "qkv"}
    b_ext = {m: nc.dram_tensor(f"b_{m}", [D], F32, kind="ExternalInput").ap()
             for m in "qkv"}
    out_s = nc.dram_tensor("out_s", [S, D], F32, kind="ExternalOutput").ap()
    out_w = nc.dram_tensor("out_w", [S, S], F32, kind="ExternalOutput").ap()

    with tile.TileContext(nc) as tc:
        _body(nc, tc, x_ext, mask_ext, W_ext, b_ext, out_s, out_w)
    nc.compile()
    return nc


def _get_nc():
    if "nc" not in _cache:
        _cache["nc"] = _build_nc()
    return _cache["nc"]


def kernel(**inputs):
    x = np.ascontiguousarray(np.asarray(inputs["input"], dtype=np.float32))
    mask = np.ascontiguousarray(np.asarray(inputs["mask"], dtype=np.int32))
    Ws = {m: np.ascontiguousarray(np.asarray(inputs[f"W_{m}"], dtype=np.float32))
          for m in "qkv"}
    bs = {m: np.ascontiguousarray(np.asarray(inputs[f"b_{m}"], dtype=np.float32))
          for m in "qkv"}

    nc = _get_nc()
    in_maps = []
    for i in range(N_CORES):
        m = {"input": x[i], "mask": mask[i]}
        for k in "qkv":
            m[f"W_{k}"] = Ws[k]
            m[f"b_{k}"] = bs[k]
        in_maps.append(m)

    trace = bool(os.environ.get("KERNEL_TRACE"))
    res = run_bass_kernel_spmd(
        nc, in_maps, core_ids=list(range(N_CORES)), trace=trace
    )
    LAST["exec_time_ns"] = res.exec_time_ns
    LAST["results"] = res
    Sout = np.stack([np.asarray(res.results[i]["out_s"]) for i in range(N_CORES)])
    Wout = np.stack([np.asarray(res.results[i]["out_w"]) for i in range(N_CORES)])
    return (Sout, Wout)


if __name__ == "__main__":
    rng = np.random.default_rng(0)
    ins = {
        "input": rng.standard_normal((B, S, D), dtype=np.float32),
        "mask": rng.integers(0, 2, size=(B, S, S), dtype=np.int32),
    }
    for m in "qkv":
        ins[f"W_{m}"] = rng.standard_normal((D, D), dtype=np.float32) / math.sqrt(D)
        ins[f"b_{m}"] = 0.01 * rng.standard_normal((D,), dtype=np.float32)
    out = kernel(**ins)
    print("Sout", out[0].shape, "W", out[1].shape)
